# revision 60
# baseline (speedup 1.0000x reference)
"""SIGFormer sparse-attention kernel for 8 Trainium2 NeuronCores.

Strategy (edge-parallel, destination-sharded), v3:
  - Each core owns a contiguous range of destination rows (N/8) and all edges
    targeting them.  Host sorts edges by destination row and packs them into
    64-row windows / 128-edge sub-chunks (padded so all 8 cores run ONE SPMD
    program; per-core differences live in the input data).
  - LayerNorm is SHARDED: each core normalizes only its own N/8 rows (host
    pre-slices embs), then two chunked AllGather collectives assemble the
    full bf16 x.  x rows are stored chunk-major across cores so chunk 0's
    output is exactly the int16 lo view [0, 32768): lo-stream gathers depend
    only on the first (earlier) collective.  xw (own dest rows, pair-packed)
    comes from the local shard via a regular DMA - no gather.
  - Per-edge source rows x[col] are fetched with dma_gather in <=768-index
    calls round-robined over the 4 SWDGE queues; slots are sorted by source
    row (HBM locality), padded slots carry idx -1 and the per-core real
    count rides in registers so desc-gen and the DMA skip pads entirely.
  - Scores: dest-row replicated matrix Xr is built with a host-precomputed
    one-hot matmul (rtp, fp8, pair-packed 2 windows per 128 partitions); the
    per-edge dot is a fused DVE mult+accumulate per sub-chunk.
  - Masking trick: host ships mp = (onehot-1)*128 (values 0/-128).  The
    aggregation lhsT a-half is built by ONE ACT op per sub-chunk:
    exp(mp + score) == onehot * exp(score) (non-own rows underflow to 0);
    the p-half is identity(mp * 2^-7 + 1) == raw one-hot (exact), alternating
    DVE/ACT per window since both engines are near-saturated.
  - Aggregation: U|V = lhsT^T @ gt accumulated over sub-chunks in PSUM;
    normalizers A|P from a ones-column matmul; reciprocal on DVE, scaling on
    ACT, constant selector matmul combines U/A + V/P.
  - The window loop is software-pipelined: window w's score matmuls/stt/exp
    interleave sub-chunk-wise with window w-1's aggregation matmuls;
    gathers/metadata loads run three windows ahead.
"""

import os
import numpy as np
import ml_dtypes

BF16 = np.dtype(ml_dtypes.bfloat16)

D = 256          # embedding dim
WROWS = 64       # destination rows per window
KE = 128         # edges per sub-chunk (matmul contraction)
VS = 32768       # rows addressable by int16 gather indices
PAD_SLOT = 64    # rowslot sentinel for pad slots (never matches 0..63)

LAST_RESULTS = None  # BassKernelResults of the last kernel() call (for test.py)
LAST_NC = None       # compiled Bass module of the last kernel() call


# ----------------------------------------------------------------------------
# Host-side packing
# ----------------------------------------------------------------------------

def _wrap16(idx_vec):
    """int16 index vector (len % 16 == 0) -> [16, len/16] layout wrapped so
    that index i lands at [i % 16, i // 16]."""
    n = idx_vec.shape[0]
    assert n % 16 == 0
    return np.ascontiguousarray(idx_vec.reshape(n // 16, 16).T)


def _pack(N, row, col, epv, n_cores):
    """Sort/partition edges and build the uniform per-core input arrays."""
    assert N % n_cores == 0, (N, n_cores)
    RPC = N // n_cores
    NW = (RPC + WROWS - 1) // WROWS
    assert NW % 2 == 0, NW
    NPAIR = NW // 2
    vs = min(VS, N)
    hi_off = max(0, N - VS)

    order = np.argsort(row, kind="stable")
    rs = row[order]
    cs = col[order]
    es = epv[order]

    # x is stored CHUNK-MAJOR across cores (all cores' chunk-0 rows first) so
    # the two AllGather chunks write contiguous ranges; col indices are
    # remapped through the same permutation.  Chunk 0 is sized so its output
    # is EXACTLY the int16 lo view [0, vs): lo-stream gathers then depend only
    # on the first (smaller, earlier) collective.
    assert vs % n_cores == 0 and (vs // n_cores) % 512 == 0
    HCH = vs // n_cores
    gc_ = cs // RPC
    gr_ = cs - gc_ * RPC
    cs = np.where(gr_ < HCH, gc_ * HCH + gr_,
                  n_cores * HCH + gc_ * (RPC - HCH) + (gr_ - HCH))

    core_id = rs // RPC
    lrow = rs - core_id * RPC          # row local to core
    wid = lrow // WROWS                # window id within core
    wrow = lrow - wid * WROWS          # row within window (0..63)
    is_lo = cs < vs

    g = core_id * NW + wid
    nlo = np.bincount(g[is_lo], minlength=n_cores * NW).reshape(n_cores, NW)
    nhi = np.bincount(g[~is_lo], minlength=n_cores * NW).reshape(n_cores, NW)

    def pad128(v):
        return ((v + KE - 1) // KE) * KE

    NLO = pad128(nlo.max(axis=0))      # [NW] padded lo slots per window
    NHI = pad128(nhi.max(axis=0))      # [NW] padded hi slots per window
    nsub_lo = NLO // KE
    nsub_hi = NHI // KE
    nsub_w = nsub_lo + nsub_hi
    NSUB = int(nsub_w.sum())
    slot_w = NLO + NHI                 # slots per window
    TOTSLOT = int(slot_w.sum())
    sub_off = np.concatenate([[0], np.cumsum(nsub_w)])      # per window
    slot_off = np.concatenate([[0], np.cumsum(slot_w)])     # per window

    nsp = np.maximum(nsub_w[0::2], nsub_w[1::2])            # per pair
    NSUBP = int(nsp.sum())
    pair_off = np.concatenate([[0], np.cumsum(nsp)])

    RPCP = ((RPC + 127) // 128) * 128  # padded own-shard rows (tail dup)
    XWN = 128 + pad128(RPC)            # xw positions (first 128 sacrificial)

    # gathers are split into <=512-index calls so each call's descriptor burst
    # (~4.4 descs/row) fits the per-queue SWDGE ring: the Pool engine then
    # issues ahead across the 4 queues and their drains overlap instead of
    # serializing.  Padded slots carry idx -1 and the per-core real count of
    # each call rides in a register so desc-gen/DMA skip the pads.
    GSEG = 768
    segs = []  # per window: list of (nidx, islo, slot_base, i16_abs, cnt_idx)
    nseg_tot = 0
    for w in range(NW):
        sw = []
        base = 0
        for n_str, islo in ((int(NLO[w]), True), (int(NHI[w]), False)):
            off = 0
            while off < n_str:
                sz = min(GSEG, n_str - off)
                sw.append((sz, islo, base + off,
                           (slot_off[w] + base + off) // 16, nseg_tot))
                nseg_tot += 1
                off += sz
            base += n_str
        segs.append(sw)

    plan = dict(
        N=N, n_cores=n_cores, RPC=RPC, RPCP=RPCP, NW=NW, NPAIR=NPAIR, vs=vs,
        hi_off=hi_off, NLO=NLO, NHI=NHI, nsub_w=nsub_w, NSUB=NSUB,
        sub_off=sub_off, slot_off=slot_off, TOTSLOT=TOTSLOT,
        nsp=nsp, NSUBP=NSUBP, pair_off=pair_off, XWN=XWN, segs=segs,
        nseg_tot=nseg_tot, HCH=HCH,
    )

    iota64 = np.arange(WROWS, dtype=np.int32)
    uniform_epv = bool(np.allclose(epv, 1.0))
    plan["uniform_epv"] = uniform_epv
    per_core = []
    for c in range(n_cores):
        m = core_id == c
        c_w = wid[m]
        c_wr = wrow[m]
        c_cs = cs[m]
        c_es = es[m]
        c_lo = is_lo[m]

        slots_idx = np.full(TOTSLOT, -1, np.int16)   # pads: idx -1 (skipped)
        slots_wr = np.full(TOTSLOT, PAD_SLOT, np.int32)
        slots_ep = np.zeros(TOTSLOT, np.float32)
        cnts = np.zeros(max(nseg_tot, 1), np.int32)

        for w in range(NW):
            wm = c_w == w
            base = slot_off[w]
            kk = {}
            for sel, cnt_max, roff, islo in (
                (wm & c_lo, NLO[w], 0, True),
                (wm & ~c_lo, NHI[w], NLO[w], False),
            ):
                k = int(sel.sum())
                assert k <= cnt_max
                kk[islo] = k
                dst = base + roff
                if cnt_max == 0 or k == 0:
                    continue
                cvals = c_cs[sel]
                if roff:  # hi stream indices are relative to the hi view
                    cvals = cvals - hi_off
                # ascending-address gathers are gentler on HBM row buffers
                srt = np.argsort(cvals, kind="stable")
                slots_idx[dst:dst + k] = cvals[srt].astype(np.int16)
                slots_wr[dst:dst + k] = c_wr[sel][srt]
                slots_ep[dst:dst + k] = c_es[sel][srt]
            for sz, islo, sbase, _, ci in segs[w]:
                soff = sbase - (0 if islo else int(NLO[w]))
                cseg = min(max(kk[islo] - soff, 0), sz)
                if cseg == 0:  # never hand the ucode a zero-count gather
                    slots_idx[base + sbase] = 0
                    cseg = 1
                cnts[ci] = cseg

        # compact mask tensors.  mp = (onehot - 1) * 128 (values 0 / -128):
        # the device derives the p-half via identity(mp * 2^-7 + 1) and the
        # a-half via exp(mp + score) (non-own rows underflow to 0).
        wr_sub = slots_wr.reshape(NSUB, KE)                  # [NSUB, 128]
        ep_sub = slots_ep.reshape(NSUB, KE)
        eq = (wr_sub[:, :, None] == iota64[None, None, :])   # [NSUB, 128, 64]
        mp = ((eq.astype(np.float32) - 1.0) * 128.0).astype(BF16)
        mp = np.ascontiguousarray(mp.transpose(1, 0, 2))     # [128, NSUB, 64]
        if uniform_epv:
            m2 = None  # p-half derived on-device from mp (saves DMA)
        else:
            m2 = (eq * ep_sub[:, :, None]).astype(BF16)
            m2 = np.ascontiguousarray(m2.transpose(1, 0, 2))  # [128, NSUB, 64]

        # rtp: per-window transposed one-hot, zero-padded to K=128 so the
        # score matmul's weight load qualifies for Fast Weight Load.
        # fp8e4m3 holds 0/1 exactly.
        rtp = np.zeros((2 * WROWS, NSUB, KE), np.dtype(ml_dtypes.float8_e4m3))
        eqT = eq.transpose(0, 2, 1)                          # [NSUB, 64, 128]
        for w in range(NW):
            half = w % 2
            s0, ns = sub_off[w], nsub_w[w]
            rtp[64 * half:64 * half + 64,
                s0:s0 + ns, :] = eqT[s0:s0 + ns].transpose(1, 0, 2)
        rtp = np.ascontiguousarray(rtp)

        # gather index table, 16-wrapped per gather block, tiled to 128 parts
        blocks = []
        for w in range(NW):
            base = slot_off[w]
            if NLO[w]:
                blocks.append(_wrap16(slots_idx[base:base + NLO[w]]))
            if NHI[w]:
                blocks.append(_wrap16(slots_idx[base + NLO[w]:base + NLO[w] + NHI[w]]))
        idx16 = np.concatenate(blocks, axis=1) if blocks else np.zeros((16, 0), np.int16)
        idx_all = np.tile(idx16, (8, 1))

        entry = dict(
            idx_all=np.ascontiguousarray(idx_all),
            mp=mp,
            rtp=rtp,
            cnts=np.ascontiguousarray(cnts[None, :]),
        )
        if m2 is not None:
            entry["m2"] = m2
        per_core.append(entry)

    return plan, per_core


# ----------------------------------------------------------------------------
# Device program
# ----------------------------------------------------------------------------

def _build_nc(plan):
    import concourse.bass as bass
    import concourse.bacc as bacc
    import concourse.tile as tile
    import concourse.mybir as mybir
    from contextlib import ExitStack

    f32 = mybir.dt.float32
    bf16 = mybir.dt.bfloat16
    fp8 = mybir.dt.float8e4
    i16 = mybir.dt.int16
    i32 = mybir.dt.int32
    EXP = mybir.ActivationFunctionType.Exp
    SQRT = mybir.ActivationFunctionType.Sqrt
    IDN = mybir.ActivationFunctionType.Identity
    CPY = mybir.ActivationFunctionType.Copy
    OP = mybir.AluOpType
    AX = mybir.AxisListType

    N = plan["N"]
    RPC = plan["RPC"]
    NW = plan["NW"]
    vs = plan["vs"]
    hi_off = plan["hi_off"]
    NLO, NHI = plan["NLO"], plan["NHI"]
    nsub_w = plan["nsub_w"]
    sub_off = plan["sub_off"]
    NSUB = plan["NSUB"]
    nsp = plan["nsp"]
    NSUBP = plan["NSUBP"]
    pair_off = plan["pair_off"]
    XWN = plan["XWN"]
    TOT16 = sum(int(NLO[w] + NHI[w]) for w in range(NW)) // 16
    XW16 = XWN // 16
    XWSLOT = XWN // 128
    NSMAX = int(nsub_w.max())
    NSPMAX = int(nsp.max())
    scale = 1.0 / float(np.sqrt(D))
    SB = 2  # score sub-chunks per DVE batch (PSUM bank budget bound)

    nc = bacc.Bacc("TRN2", target_bir_lowering=False, debug=False,
                   num_swdge_queues=4, dynamic_dma_scratch_size=49152)

    uniform_epv = plan["uniform_epv"]
    RPCP = plan["RPCP"]
    n_cores = plan["n_cores"]
    embs_mine = nc.dram_tensor("embs_mine", [RPCP, D], f32,
                               kind="ExternalInput").ap()
    idx_all_d = nc.dram_tensor("idx_all", [128, max(TOT16, 1)], i16,
                               kind="ExternalInput").ap()
    mp_d = nc.dram_tensor("mp", [128, NSUB, WROWS], bf16,
                          kind="ExternalInput").ap()
    m2_d = None if uniform_epv else nc.dram_tensor(
        "m2", [128, NSUB, WROWS], bf16, kind="ExternalInput").ap()
    rtp_d = nc.dram_tensor("rtp", [128, NSUB, 128], fp8,
                           kind="ExternalInput").ap()
    NSEG_TOT = max(int(plan["nseg_tot"]), 1)
    cnts_d = nc.dram_tensor("cnts", [1, NSEG_TOT], i32,
                            kind="ExternalInput").ap()
    out_d = nc.dram_tensor("out", [RPC, D], f32, kind="ExternalOutput").ap()
    x_my = nc.dram_tensor("x_my", [RPCP, D], bf16, kind="Internal").ap()
    x_d = nc.dram_tensor("x", [N, D], bf16, kind="Internal",
                         addr_space="Shared").ap()

    with ExitStack() as ctx:
        tc = ctx.enter_context(tile.TileContext(nc))

        consts = ctx.enter_context(tc.tile_pool(name="consts", bufs=1))
        gpool = ctx.enter_context(tc.tile_pool(name="gath", bufs=5))
        rtpool = ctx.enter_context(tc.tile_pool(name="rt", bufs=4))
        ltpool = ctx.enter_context(tc.tile_pool(name="lt", bufs=4))
        mppool = ctx.enter_context(tc.tile_pool(name="mpp", bufs=4))
        spool = ctx.enter_context(tc.tile_pool(name="small", bufs=4))
        prodp = ctx.enter_context(tc.tile_pool(name="prod", bufs=3))
        otpool = ctx.enter_context(tc.tile_pool(name="ot", bufs=2))
        osbp = ctx.enter_context(tc.tile_pool(name="osb", bufs=2))
        ps_xr = ctx.enter_context(tc.tile_pool(name="ps_xr", bufs=4, space="PSUM"))
        ps_uv = ctx.enter_context(tc.tile_pool(name="ps_uv", bufs=2, space="PSUM"))
        ps_ap = ctx.enter_context(tc.tile_pool(name="ps_ap", bufs=1, space="PSUM"))
        ps_fin = ctx.enter_context(tc.tile_pool(name="ps_fin", bufs=1, space="PSUM"))
        ln_ctx = ExitStack()
        lnpool = ln_ctx.enter_context(tc.tile_pool(name="ln", bufs=3))
        lnout = ln_ctx.enter_context(tc.tile_pool(name="lnout", bufs=3))
        lnstat = ln_ctx.enter_context(tc.tile_pool(name="lnstat", bufs=4))

        # ---- preload metadata -------------------------------------------------
        sb_idx = consts.tile([128, max(TOT16, 1)], i16)
        nc.sync.dma_start(sb_idx, idx_all_d)
        sb_cnt = consts.tile([1, NSEG_TOT], i32)
        nc.sync.dma_start(sb_cnt, cnts_d)

        # ---- constant tiles ---------------------------------------------------
        iota64 = consts.tile([128, WROWS], f32)
        nc.gpsimd.iota(iota64, pattern=[[1, WROWS]], base=0, channel_multiplier=0,
                       allow_small_or_imprecise_dtypes=True)
        pidx = consts.tile([128, 1], f32)
        nc.gpsimd.iota(pidx, pattern=[[1, 1]], base=0, channel_multiplier=1,
                       allow_small_or_imprecise_dtypes=True)
        ones_bf = consts.tile([128, 1], bf16)
        nc.vector.memset(ones_bf, 1.0)
        epsc = consts.tile([128, 1], f32)
        nc.vector.memset(epsc, 1e-5)
        # mc[p, r] = 1 if p % 64 == r  (bf16 selector for U+V combine)
        pidx2 = consts.tile([128, 1], f32)
        nc.vector.tensor_scalar(pidx2, pidx, 64.0, None, OP.subtract)
        mc = consts.tile([128, WROWS], bf16)
        nc.vector.tensor_scalar(mc, iota64, pidx, None, OP.is_equal)
        nc.vector.scalar_tensor_tensor(mc, iota64, pidx2, mc, OP.is_equal, OP.add)

        from concourse import library_config

        # consts above used the default gpsimd library (iota); fence, then
        # switch to mlp (hosts DMAGatherAnt) while LN runs on DVE/ACT.
        tc.strict_bb_all_engine_barrier()
        nc.gpsimd.load_library(library_config.mlp)
        tc.no_sync_barrier()

        # ---- phase 1: LayerNorm of OWN rows only -> x_my (bf16) ---------------
        JR = 4  # node rows per partition
        TROWS = 128 * JR
        n_full = RPCP // TROWS
        tail = RPCP - n_full * TROWS
        assert tail % JR == 0
        e_t = embs_mine[0:n_full * TROWS].rearrange(
            "(t p j) d -> t p (j d)", p=128, j=JR) if n_full else None
        x_t = x_my[0:n_full * TROWS].rearrange(
            "(t p j) d -> t p (j d)", p=128, j=JR) if n_full else None

        def ln_tile(src_ap, dst_ap, p):
            xt = lnpool.tile([128, JR, D], f32)
            nc.sync.dma_start(xt[:p].rearrange("p j d -> p (j d)"), src_ap)
            stats = lnstat.tile([128, JR, 6], f32)
            mv = lnstat.tile([128, JR, 2], f32)
            for gji in range(JR):
                nc.vector.bn_stats(stats[:p, gji, :], xt[:p, gji, :])
                nc.vector.bn_aggr(mv[:p, gji, :], stats[:p, gji, :])
            sd = lnstat.tile([128, JR, 1], f32)
            nc.scalar.activation(sd[:p], mv[:p, :, 1:2], SQRT, bias=epsc[:p])
            rs_ = lnstat.tile([128, JR, 1], f32)
            nc.vector.reciprocal(rs_[:p], sd[:p])
            nmrs = lnstat.tile([128, JR, 1], f32)
            nc.vector.scalar_tensor_tensor(
                nmrs[:p], mv[:p, :, 0:1], -1.0, rs_[:p], OP.mult, OP.mult)
            xo = lnout.tile([128, JR, D], bf16)
            for gji in range(JR):
                nc.scalar.activation(xo[:p, gji, :], xt[:p, gji, :], IDN,
                                     scale=rs_[:p, gji, :], bias=nmrs[:p, gji, :])
            nc.sync.dma_start(dst_ap, xo[:p].rearrange("p j d -> p (j d)"))

        for t in range(n_full):
            ln_tile(e_t[t], x_t[t], 128)
        if tail:
            ln_tile(
                embs_mine[n_full * TROWS:].rearrange("(p j) d -> p (j d)", j=JR),
                x_my[n_full * TROWS:].rearrange("(p j) d -> p (j d)", j=JR),
                tail // JR)
        ln_ctx.close()

        x_lo = x_d[0:vs, :]
        x_hi = x_d[hi_off:hi_off + vs, :]

        # ---- phase 1b: xw from own shard (regular DMA, no gather) -------------
        xw = consts.tile([128, XWSLOT, D], bf16)
        nc.sync.dma_start(
            xw[:, 1:XWSLOT, :],
            x_my[0:(XWSLOT - 1) * 128].rearrange("(s p) d -> p s d", p=128))

        # ---- phase 2: all-gather x shards into the full x ---------------------
        # x is chunk-major across cores, so each chunk's AllGather output is a
        # contiguous slab; chunk 0 ships while LN finishes the tail rows.
        HCH = plan["HCH"]
        nc.gpsimd.collective_compute(
            "AllGather", OP.bypass,
            replica_groups=[list(range(n_cores))],
            ins=[x_my[0:HCH, :]],
            outs=[x_d[0:n_cores * HCH, :]],
        )
        nc.gpsimd.collective_compute(
            "AllGather", OP.bypass,
            replica_groups=[list(range(n_cores))],
            ins=[x_my[HCH:RPC, :]],
            outs=[x_d[n_cores * HCH:n_cores * RPC, :]],
        )

        segs = plan["segs"]
        cregs = [ctx.enter_context(nc.gpsimd.register(f"cnt{i}"))
                 for i in range(8)]
        creg_i = 0

        gq = 0          # SWDGE queue round-robin
        rtp_tiles = {}
        state = {}

        def stage_load_meta(w):
            ns = int(nsub_w[w])
            p = w // 2
            if ns == 0:
                return
            s0 = int(sub_off[w])
            rtp_t = rtpool.tile([128, NSMAX, KE], fp8, tag="rtp")
            nc.sync.dma_start(rtp_t[:, 0:ns, :], rtp_d[:, s0:s0 + ns, :])
            rtp_tiles[w] = rtp_t
            lt_t = ltpool.tile([128, NSMAX, 2 * WROWS], bf16, tag="lt")
            mp_t = mppool.tile([128, NSMAX, WROWS], bf16, tag="mp")
            nc.sync.dma_start(mp_t[:, 0:ns, :], mp_d[:, s0:s0 + ns, :])
            if not uniform_epv:
                nc.sync.dma_start(lt_t[:, 0:ns, WROWS:], m2_d[:, s0:s0 + ns, :])
            gt = gpool.tile([128, NSMAX, D], bf16, tag="gt")
            if w < 5:  # first use of each gather buf: clear pad-slot garbage
                nc.vector.memset(gt, 0.0)
            state[w] = [lt_t, gt, mp_t]

        def stage_load_gather(w):
            nonlocal gq, creg_i
            sw = segs[w]
            if int(nsub_w[w]) == 0 or not sw:
                return
            gt = state[w][1]
            k = len(sw)
            ci0 = sw[0][4]
            rs = [cregs[(creg_i + i) % len(cregs)] for i in range(k)]
            creg_i = (creg_i + k) % len(cregs)
            nc.gpsimd.reg_load(rs, sb_cnt[0:1, ci0:ci0 + k])
            for (sz, islo, sbase, i16a, ci), r in zip(sw, rs):
                nc.gpsimd.dma_gather(
                    gt[:, sbase // KE: (sbase + sz) // KE, :],
                    x_lo if islo else x_hi,
                    sb_idx[:, i16a: i16a + sz // 16],
                    sz, r, D, elem_step=D, single_packet=False,
                    queue_num=gq)
                gq = (gq + 1) % 4

        NPRE = min(3, NW)
        for w0 in range(NPRE):
            stage_load_meta(w0)
            stage_load_gather(w0)

        # ---- phase 3: windows -------------------------------------------------
        # Software pipeline, sub-chunk interleaved: gathers/loads are issued
        # three windows ahead; the fused j-loop alternates window w's score
        # matmul+stt with window w-1's aggregation matmuls so the PE has agg
        # work to run while the DVE paces through the score dot-products.

        def fused(w):
            """Scores for window w interleaved with aggregation for w-1."""
            ns = int(nsub_w[w]) if 0 <= w < NW else 0
            nsb = int(nsub_w[w - 1]) if w >= 1 else 0
            if ns:
                xw_rhs = xw[:, 1 + w // 2, :]
                rtp_t = rtp_tiles.pop(w)
                lt_t, gt, mp_t = state[w]
                sc_w = spool.tile([128, NSMAX, 1], f32, tag="sc")
                if uniform_epv:
                    # p-half: (mp * 2^-7) + 1 == raw one-hot (exact).
                    # Alternate engines: DVE and ACT are both near-saturated.
                    if w % 2 == 0:
                        nc.vector.tensor_scalar(lt_t[:, 0:ns, WROWS:],
                                                mp_t[:, 0:ns, :], 0.0078125,
                                                1.0, OP.mult, OP.add)
                    else:
                        nc.scalar.activation(lt_t[:, 0:ns, WROWS:],
                                             mp_t[:, 0:ns, :], IDN,
                                             scale=0.0078125, bias=1.0)
            if nsb:
                lt_b, gt_b = state[w - 1][0], state[w - 1][1]
                puv = ps_uv.tile([128, D], f32)
                pap = ps_ap.tile([128, 1], f32, tag="ap")

            for j in range(max(ns, nsb)):
                if j < ns:
                    pxr = ps_xr.tile([128, D], f32)
                    nc.tensor.matmul(pxr, rtp_t[:, j, :], xw_rhs,
                                     start=True, stop=True)
                if j < nsb:
                    nc.tensor.matmul(puv, lt_b[:, j, :], gt_b[:, j, :],
                                     start=(j == 0), stop=(j == nsb - 1))
                    nc.tensor.matmul(pap, lt_b[:, j, :], ones_bf,
                                     start=(j == 0), stop=(j == nsb - 1))
                if j < ns:
                    prod = prodp.tile([128, D], bf16, tag="prod")
                    nc.vector.scalar_tensor_tensor(
                        prod, gt[:, j, :], scale, pxr, OP.mult, OP.mult,
                        accum_out=sc_w[:, j, :])
                    # a-half: exp(mp + s) = onehot * exp(s) (else underflow->0)
                    nc.scalar.activation(lt_t[:, j, 0:WROWS], mp_t[:, j, :],
                                         EXP, scale=1.0, bias=sc_w[:, j, :])

            if w >= 1:
                wb = w - 1
                wr = min(WROWS, RPC - WROWS * wb)
                osb = osbp.tile([64, D], f32)
                if nsb:
                    ssum = spool.tile([128, 1], f32, tag="ssum")
                    nc.vector.tensor_scalar(ssum, pap, 1e-30, None, OP.add)
                    rap = spool.tile([128, 1], f32, tag="rap")
                    nc.vector.reciprocal(rap, ssum)
                    ot = otpool.tile([128, D], bf16)
                    nc.scalar.activation(ot, puv, IDN, scale=rap)
                    pfin = ps_fin.tile([64, D], f32, tag="fin")
                    nc.tensor.matmul(pfin, mc, ot, start=True, stop=True)
                    nc.scalar.activation(osb[:wr], pfin[:wr], CPY)
                    state.pop(w - 1)
                else:
                    nc.vector.memset(osb[:wr], 0.0)
                nc.sync.dma_start(out_d[WROWS * wb: WROWS * wb + wr, :],
                                  osb[:wr])

        for w in range(NW + 1):
            if w + 3 < NW:
                stage_load_meta(w + 3)
                stage_load_gather(w + 3)
            fused(w)

    nc.compile()
    return nc


# ----------------------------------------------------------------------------
# Entry point
# ----------------------------------------------------------------------------

def _numpy_reference(embs, SSE, SPE, path_emb, spec_lambda):
    """Fallback (only used when spec_lambda != 0, which the problem spec's
    input fills never produce)."""
    x = embs - embs.mean(-1, keepdims=True)
    x = x / np.sqrt((x * x).mean(-1, keepdims=True) + 1e-5)
    row, col, ptyp = SPE[:, 0], SPE[:, 1], SPE[:, 2]
    n = embs.shape[0]
    a = (x[row] * x[col]).sum(-1) / np.sqrt(np.float32(D))
    a = a + np.float32(spec_lambda[0]) * (SSE[row] * SSE[col]).sum(-1)

    def seg_softmax(v, seg):
        mx = np.full(n, -np.inf, np.float32)
        np.maximum.at(mx, seg, v)
        mx[~np.isfinite(mx)] = 0.0
        e = np.exp(v - mx[seg])
        sm = np.zeros(n, np.float32)
        np.add.at(sm, seg, e)
        return e / sm[seg]

    p = path_emb[ptyp].reshape(-1)
    wgt = seg_softmax(a, row) + seg_softmax(p, row)
    out = np.zeros_like(x)
    np.add.at(out, row, wgt[:, None] * x[col])
    return out


def kernel(n_cores=8, **inputs):
    global LAST_RESULTS
    embs = np.ascontiguousarray(np.asarray(inputs["embs"], dtype=np.float32))
    SSE = np.asarray(inputs["SSE"], dtype=np.float32)
    SPE = np.asarray(inputs["SPE"])
    path_emb = np.asarray(inputs["path_emb"], dtype=np.float32)
    spec_lambda = np.asarray(inputs["spec_lambda"], dtype=np.float32)

    if float(np.abs(spec_lambda).max()) != 0.0:
        return _numpy_reference(embs, SSE, SPE, path_emb, spec_lambda)

    N = embs.shape[0]
    row = np.asarray(SPE[:, 0], dtype=np.int64)
    col = np.asarray(SPE[:, 1], dtype=np.int64)
    ptyp = np.asarray(SPE[:, 2], dtype=np.int64)
    epv = np.exp(path_emb.astype(np.float32).reshape(-1))[ptyp]

    plan, per_core = _pack(N, row, col, epv, n_cores)
    nc = _build_nc(plan)
    global LAST_NC
    LAST_NC = nc

    from concourse import bass_utils
    RPC, RPCP = plan["RPC"], plan["RPCP"]
    in_maps = []
    for c in range(n_cores):
        m = dict(per_core[c])
        mine = embs[c * RPC:(c + 1) * RPC]
        if RPCP > RPC:
            mine = np.concatenate(
                [mine, np.broadcast_to(mine[-1], (RPCP - RPC, D))], axis=0)
        m["embs_mine"] = np.ascontiguousarray(mine)
        in_maps.append(m)
    want_trace = bool(int(os.environ.get("KERNEL_TRACE", "0")))
    if want_trace:
        try:  # tracing needs the NTFF hook; never let its absence crash a run
            import antenv.axon_hooks  # noqa: F401
        except ImportError:
            want_trace = False
    res = bass_utils.run_bass_kernel_spmd(
        nc, in_maps, core_ids=list(range(n_cores)), trace=want_trace,
    )
    LAST_RESULTS = res
    out = np.concatenate([res.results[c]["out"] for c in range(n_cores)], axis=0)
    return out[:N].astype(np.float32)

